# revision 19
# baseline (speedup 1.0000x reference)
"""Trainium2 Bass kernel for nn_Network_54073638257187 (ragged_sequence).

Math (collapsed from the reference):
    A[b,t] = hidden[b,t,:] @ fc_w          (per-token scalar projection)
    E[b,t] = hidden[b,t,:] @ emo_w
    For each (doc b, clause j) with start s and length L:
        a_k = A[b, s+k] + (fc_b if k < L else -9e5)     k = 0..63
        t_k = exp(a_k - max_k a_k)
        pred[b,j] = sigmoid( (sum_k t_k * E[b, s+k]) / (sum_k t_k) + emo_b )

The only heavy part is the two mat-vec projections over the 402MB
hidden_states tensor -> done on the TensorEngine from a host-transposed
[D, tokens] layout so DMA streams contiguously at line rate.  The ragged
"gather of clauses" operates on per-token *scalars* (contiguous 64-float
windows), fetched with an indirect DMA.  Only tokens up to the last
clause start + 64 are ever referenced, so the token axis is trimmed to
T_eff (data-dependent, rounded to 512) before upload.

Sharding: pure data parallelism -- 4 docs per core across 8 cores.
"""

import numpy as np
from contextlib import ExitStack

import concourse.bass as bass
import concourse.bacc as bacc
import concourse.tile as tile
from concourse import mybir
from concourse.bass_utils import run_bass_kernel_spmd

NEG = -900000.0
P = 128
DCH = 6            # d chunks (768 / 128)
QN = 512           # tokens per matmul / psum group
SG = 1024          # tokens per DMA supergroup tile
NCORES = 8
DPC = 4            # docs per core
J = 64             # clauses per doc
K = 64             # tokens per clause
USE_FP32R = False   # fp32r self-loading matmuls crash TRN2 (NRT status 101)


def _emit_kernel(nc, NT, fcb, emb, use_fp32r):
    """Build the per-core program. NT = DPC * T_eff tokens on this core."""
    f32 = mybir.dt.float32
    NA = NT + K
    n_sg = NT // SG
    rem = NT - n_sg * SG          # leftover tokens (multiple of QN)

    hdt = mybir.dt.float32r if use_fp32r else f32
    ht = nc.dram_tensor("ht", [DCH, P, NT], f32, kind="ExternalInput").ap()
    w2 = nc.dram_tensor("w2", [DCH, P, 2], f32, kind="ExternalInput").ap()
    woff = nc.dram_tensor("woff", [2, P, 1], mybir.dt.int32, kind="ExternalInput").ap()
    maskadd = nc.dram_tensor("maskadd", [2, P, K], f32, kind="ExternalInput").ap()
    out = nc.dram_tensor("out", [2, P], f32, kind="ExternalOutput").ap()

    A_d = nc.dram_tensor("A_scr", [NA, 1], f32).ap()
    E_d = nc.dram_tensor("E_scr", [NA, 1], f32).ap()

    with tile.TileContext(nc) as tc, ExitStack() as ctx:
        consts = ctx.enter_context(tc.tile_pool(name="consts", bufs=1))
        loads = ctx.enter_context(tc.tile_pool(name="loads", bufs=2))
        psum = ctx.enter_context(tc.tile_pool(name="psum", bufs=8, space="PSUM"))
        stage = ctx.enter_context(tc.tile_pool(name="stage", bufs=8))
        p2 = ctx.enter_context(tc.tile_pool(name="p2", bufs=2))

        # ---- constants ----
        # Matmuls may carry at most ONE HW sync wait (S3_LW slot), so every
        # cross-engine dependency of a matmul is routed through the Vector
        # semaphore: the fp32r rounding-gate copy, the w2 staging copy and
        # the PSUM evacuation all run on DVE.
        w2st = consts.tile([P, DCH, 2], f32)
        nc.gpsimd.dma_start(out=w2st[:, :, :], in_=w2.rearrange("c p m -> p c m"))
        w2sb = consts.tile([P, DCH, 2], hdt)
        nc.vector.tensor_copy(w2sb[:, :, :], w2st[:, :, :])
        zpad = consts.tile([1, K], f32)
        nc.vector.memset(zpad[:, :], 0.0)
        nc.scalar.dma_start(out=A_d[NT:NA, :], in_=zpad[:1, :])
        nc.scalar.dma_start(out=E_d[NT:NA, :], in_=zpad[:1, :])

        # ---- phase 1: stream ht, project onto (fc_w, emo_w) via PE ----
        def do_group(htile, q, col0):
            pt = psum.tile([2, QN], f32)
            for c in range(DCH):
                nc.tensor.matmul(out=pt[:, :], lhsT=w2sb[:, c, :],
                                 rhs=htile[:, c, q * QN:(q + 1) * QN],
                                 start=(c == 0), stop=(c == DCH - 1))
            ae = stage.tile([2, QN], f32)
            nc.vector.tensor_copy(ae[:, :], pt[:, :])
            nc.scalar.dma_start(out=A_d[col0:col0 + QN, :], in_=ae[0:1, :])
            nc.scalar.dma_start(out=E_d[col0:col0 + QN, :], in_=ae[1:2, :])

        ht_p = ht.rearrange("c p t -> p c t")          # [128, 6, NT] view
        def do_sg(col0, ncols):
            raw = loads.tile([P, DCH, SG], f32, tag="raw")
            nc.gpsimd.dma_start(out=raw[:, :, :ncols],
                                in_=ht_p[:, :, col0:col0 + ncols])
            htile = loads.tile([P, DCH, SG], hdt, tag="ht")
            nc.vector.tensor_copy(htile[:, :, :ncols], raw[:, :, :ncols])
            for q in range(ncols // QN):
                do_group(htile, q, col0 + q * QN)

        for sg in range(n_sg):
            do_sg(sg * SG, SG)
        if rem:
            do_sg(n_sg * SG, rem)

        # ---- phase 2: windowed gather on scalars + masked softmax ----
        for t in range(2):
            offs = p2.tile([P, 1], mybir.dt.int32, tag="offs")
            nc.gpsimd.dma_start(out=offs[:, :], in_=woff[t])
            aw = p2.tile([P, K], f32, tag="aw")
            nc.gpsimd.indirect_dma_start(
                out=aw[:, :], out_offset=None, in_=A_d[:, :],
                in_offset=bass.IndirectOffsetOnAxis(ap=offs[:, :1], axis=0))
            ew = p2.tile([P, K], f32, tag="ew")
            nc.gpsimd.indirect_dma_start(
                out=ew[:, :], out_offset=None, in_=E_d[:, :],
                in_offset=bass.IndirectOffsetOnAxis(ap=offs[:, :1], axis=0))
            mk = p2.tile([P, K], f32, tag="mk")
            nc.gpsimd.dma_start(out=mk[:, :], in_=maskadd[t])

            am = p2.tile([P, K], f32, tag="am")
            nc.vector.tensor_add(am[:, :], aw[:, :], mk[:, :])
            negmax = p2.tile([P, 1], f32, tag="negmax")
            nc.vector.tensor_reduce(negmax[:, :], am[:, :],
                                    axis=mybir.AxisListType.X,
                                    op=mybir.AluOpType.max, negate=True)
            tw = p2.tile([P, K], f32, tag="tw")
            ssum = p2.tile([P, 1], f32, tag="ssum")
            nc.scalar.activation(tw[:, :], am[:, :],
                                 mybir.ActivationFunctionType.Exp,
                                 bias=negmax[:, :1], scale=1.0,
                                 accum_out=ssum[:, :1])
            prod = p2.tile([P, K], f32, tag="prod")
            nsum = p2.tile([P, 1], f32, tag="nsum")
            # tensor_tensor_reduce crashes TRN2 here -- use mul + reduce
            nc.vector.tensor_mul(prod[:, :], tw[:, :], ew[:, :])
            nc.vector.reduce_sum(nsum[:, :], prod[:, :],
                                 axis=mybir.AxisListType.X)
            rec = p2.tile([P, 1], f32, tag="rec")
            nc.vector.reciprocal(rec[:, :], ssum[:, :])
            ratio = p2.tile([P, 1], f32, tag="ratio")
            nc.vector.tensor_mul(ratio[:, :], nsum[:, :], rec[:, :])
            osb = p2.tile([P, 1], f32, tag="osb")
            nc.scalar.activation(osb[:, :], ratio[:, :],
                                 mybir.ActivationFunctionType.Sigmoid,
                                 bias=float(emb), scale=1.0)
            nc.sync.dma_start(out=out[t], in_=osb[:, :])
    return nc


def _prepare(hidden_states, clause_len, fc_w, fc_b, emo_w, emo_b):
    h = np.asarray(hidden_states, dtype=np.float32)
    cl = np.asarray(clause_len).astype(np.int64)
    B, T, D = h.shape
    assert D == DCH * P and B == NCORES * DPC
    starts = np.cumsum(cl, axis=1) - cl                       # [B, J]
    need = int((starts[:, -1] + K).max())
    T_eff = -(-need // QN) * QN
    T_copy = min(T_eff, T)
    NT = DPC * T_eff

    fcb = float(np.asarray(fc_b).reshape(-1)[0])
    emb = float(np.asarray(emo_b).reshape(-1)[0])
    w2 = np.stack([np.asarray(fc_w, np.float32),
                   np.asarray(emo_w, np.float32)], axis=1)    # [768, 2]
    w2 = np.ascontiguousarray(w2.reshape(DCH, P, 2))

    tokk = np.arange(K)
    in_maps = []
    for c in range(NCORES):
        docs = slice(c * DPC, (c + 1) * DPC)
        hc = h[docs]                                          # [DPC, T, D]
        ht = np.zeros((D, DPC, T_eff), np.float32)
        ht[:, :, :T_copy] = hc[:, :T_copy, :].transpose(2, 0, 1)
        ht = np.ascontiguousarray(ht).reshape(DCH, P, NT)

        st = starts[docs]                                     # [DPC, J]
        lc = cl[docs]
        w = np.arange(2 * P)
        b_l, j_l = w // J, w % J
        woff = (b_l * T_eff + st[b_l, j_l]).astype(np.int32).reshape(2, P, 1)
        mask = np.where(tokk[None, :] < lc[b_l, j_l][:, None],
                        np.float32(fcb), np.float32(NEG)).astype(np.float32)
        maskadd = mask.reshape(2, P, K)
        in_maps.append({"ht": ht, "w2": w2, "woff": woff, "maskadd": maskadd})
    return in_maps, NT, fcb, emb


def run(inputs, trace=False, use_fp32r=USE_FP32R):
    in_maps, NT, fcb, emb = _prepare(**inputs)
    nc = bacc.Bacc(
        "TRN2", target_bir_lowering=False, debug=False, num_devices=NCORES
    )
    _emit_kernel(nc, NT, fcb, emb, use_fp32r)
    nc.compile()
    res = run_bass_kernel_spmd(nc, in_maps, core_ids=list(range(NCORES)),
                               trace=trace)
    pred = np.concatenate(
        [r["out"].reshape(2 * P).reshape(DPC, J) for r in res.results], axis=0)
    return pred.astype(np.float32), res


def kernel(**inputs):
    pred, _ = run(inputs, trace=False)
    return pred


# revision 22
# speedup vs baseline: 1.5885x; 1.5885x over previous
"""Trainium2 Bass kernel for nn_Network_54073638257187 (ragged_sequence).

Math (collapsed from the reference):
    A[b,t] = hidden[b,t,:] @ fc_w          (per-token scalar projection)
    E[b,t] = hidden[b,t,:] @ emo_w
    For each (doc b, clause j) with start s and length L:
        a_k = A[b, s+k] + (fc_b if k < L else -9e5)     k = 0..63
        t_k = exp(a_k - max_k a_k)
        pred[b,j] = sigmoid( (sum_k t_k * E[b, s+k]) / (sum_k t_k) + emo_b )

The only heavy part is the two mat-vec projections over the 402MB
hidden_states tensor -> done on the TensorEngine from a host-transposed
[D, tokens] layout so DMA streams contiguously at line rate.  The ragged
"gather of clauses" operates on per-token *scalars* (contiguous 64-float
windows), fetched with an indirect DMA.  Only tokens up to the last
clause start + 64 are ever referenced, so the token axis is trimmed to
T_eff (data-dependent, rounded to 512) before upload.

Sharding: pure data parallelism -- 4 docs per core across 8 cores.
"""

import numpy as np
from contextlib import ExitStack

import concourse.bass as bass
import concourse.bacc as bacc
import concourse.tile as tile
from concourse import mybir
from concourse.bass_utils import run_bass_kernel_spmd

NEG = -900000.0
P = 128
DCH = 6            # d chunks (768 / 128)
QN = 512           # tokens per matmul / psum group
SG = 2048          # tokens per DMA supergroup tile
NCORES = 8
DPC = 4            # docs per core
J = 64             # clauses per doc
K = 64             # tokens per clause
USE_FP32R = False   # fp32r self-loading matmuls crash TRN2 (NRT status 101)
H_DTYPE = "bf16"    # "bf16" (2x less DMA, 2.6x less PE) or "f32" (exact)


def _emit_kernel(nc, NT, fcb, emb, h_dtype=H_DTYPE):
    """Build the per-core program. NT = DPC * T_eff tokens on this core."""
    f32 = mybir.dt.float32
    NA = NT + K
    n_sg = NT // SG
    rem = NT - n_sg * SG          # leftover tokens (multiple of QN)

    hdt = mybir.dt.bfloat16 if h_dtype == "bf16" else f32
    ht = nc.dram_tensor("ht", [DCH, P, NT], hdt, kind="ExternalInput").ap()
    w2 = nc.dram_tensor("w2", [DCH, P, 2], hdt, kind="ExternalInput").ap()
    woff = nc.dram_tensor("woff", [2, P, 1], mybir.dt.int32, kind="ExternalInput").ap()
    maskadd = nc.dram_tensor("maskadd", [2, P, K], f32, kind="ExternalInput").ap()
    out = nc.dram_tensor("out", [2, P], f32, kind="ExternalOutput").ap()

    A_d = nc.dram_tensor("A_scr", [NA, 1], f32).ap()
    E_d = nc.dram_tensor("E_scr", [NA, 1], f32).ap()

    with tile.TileContext(nc) as tc, ExitStack() as ctx:
        consts = ctx.enter_context(tc.tile_pool(name="consts", bufs=1))
        loads = ctx.enter_context(tc.tile_pool(name="loads", bufs=2))
        psum = ctx.enter_context(tc.tile_pool(name="psum", bufs=8, space="PSUM"))
        stage = ctx.enter_context(tc.tile_pool(name="stage", bufs=8))
        p2 = ctx.enter_context(tc.tile_pool(name="p2", bufs=2))

        # ---- constants ----
        # Matmuls may carry at most ONE HW sync wait (S3_LW slot), so every
        # cross-engine dependency of a matmul is routed through the Vector
        # semaphore: the fp32r rounding-gate copy, the w2 staging copy and
        # the PSUM evacuation all run on DVE.
        w2st = consts.tile([P, DCH, 2], hdt)
        nc.gpsimd.dma_start(out=w2st[:, :, :], in_=w2.rearrange("c p m -> p c m"))
        w2sb = consts.tile([P, DCH, 2], hdt)
        nc.vector.tensor_copy(w2sb[:, :, :], w2st[:, :, :])
        zpad = consts.tile([1, K], f32)
        nc.vector.memset(zpad[:, :], 0.0)
        nc.scalar.dma_start(out=A_d[NT:NA, :], in_=zpad[:1, :])
        nc.scalar.dma_start(out=E_d[NT:NA, :], in_=zpad[:1, :])

        # ---- phase 1: stream ht, project onto (fc_w, emo_w) via PE ----
        def do_group(htile, q, col0):
            pt = psum.tile([2, QN], f32)
            for c in range(DCH):
                nc.tensor.matmul(out=pt[:, :], lhsT=w2sb[:, c, :],
                                 rhs=htile[:, c, q * QN:(q + 1) * QN],
                                 start=(c == 0), stop=(c == DCH - 1))
            ae = stage.tile([2, QN], f32)
            nc.vector.tensor_copy(ae[:, :], pt[:, :])
            nc.scalar.dma_start(out=A_d[col0:col0 + QN, :], in_=ae[0:1, :])
            nc.scalar.dma_start(out=E_d[col0:col0 + QN, :], in_=ae[1:2, :])

        ht_p = ht.rearrange("c p t -> p c t")          # [128, 6, NT] view
        def do_sg(col0, ncols):
            raw = loads.tile([P, DCH, SG], hdt, tag="raw")
            nc.gpsimd.dma_start(out=raw[:, :, :ncols],
                                in_=ht_p[:, :, col0:col0 + ncols])
            htile = loads.tile([P, DCH, SG], hdt, tag="ht")
            nc.vector.tensor_copy(htile[:, :, :ncols], raw[:, :, :ncols])
            for q in range(ncols // QN):
                do_group(htile, q, col0 + q * QN)

        for sg in range(n_sg):
            do_sg(sg * SG, SG)
        if rem:
            do_sg(n_sg * SG, rem)

        # ---- phase 2: windowed gather on scalars + masked softmax ----
        for t in range(2):
            offs = p2.tile([P, 1], mybir.dt.int32, tag="offs")
            nc.gpsimd.dma_start(out=offs[:, :], in_=woff[t])
            aw = p2.tile([P, K], f32, tag="aw")
            nc.gpsimd.indirect_dma_start(
                out=aw[:, :], out_offset=None, in_=A_d[:, :],
                in_offset=bass.IndirectOffsetOnAxis(ap=offs[:, :1], axis=0))
            ew = p2.tile([P, K], f32, tag="ew")
            nc.gpsimd.indirect_dma_start(
                out=ew[:, :], out_offset=None, in_=E_d[:, :],
                in_offset=bass.IndirectOffsetOnAxis(ap=offs[:, :1], axis=0))
            mk = p2.tile([P, K], f32, tag="mk")
            nc.gpsimd.dma_start(out=mk[:, :], in_=maskadd[t])

            am = p2.tile([P, K], f32, tag="am")
            nc.vector.tensor_add(am[:, :], aw[:, :], mk[:, :])
            negmax = p2.tile([P, 1], f32, tag="negmax")
            nc.vector.tensor_reduce(negmax[:, :], am[:, :],
                                    axis=mybir.AxisListType.X,
                                    op=mybir.AluOpType.max, negate=True)
            tw = p2.tile([P, K], f32, tag="tw")
            ssum = p2.tile([P, 1], f32, tag="ssum")
            nc.scalar.activation(tw[:, :], am[:, :],
                                 mybir.ActivationFunctionType.Exp,
                                 bias=negmax[:, :1], scale=1.0,
                                 accum_out=ssum[:, :1])
            prod = p2.tile([P, K], f32, tag="prod")
            nsum = p2.tile([P, 1], f32, tag="nsum")
            # tensor_tensor_reduce crashes TRN2 here -- use mul + reduce
            nc.vector.tensor_mul(prod[:, :], tw[:, :], ew[:, :])
            nc.vector.reduce_sum(nsum[:, :], prod[:, :],
                                 axis=mybir.AxisListType.X)
            rec = p2.tile([P, 1], f32, tag="rec")
            nc.vector.reciprocal(rec[:, :], ssum[:, :])
            ratio = p2.tile([P, 1], f32, tag="ratio")
            nc.vector.tensor_mul(ratio[:, :], nsum[:, :], rec[:, :])
            osb = p2.tile([P, 1], f32, tag="osb")
            nc.scalar.activation(osb[:, :], ratio[:, :],
                                 mybir.ActivationFunctionType.Sigmoid,
                                 bias=float(emb), scale=1.0)
            nc.sync.dma_start(out=out[t], in_=osb[:, :])
    return nc


def _prepare(hidden_states, clause_len, fc_w, fc_b, emo_w, emo_b,
             h_dtype=H_DTYPE):
    import ml_dtypes
    np_hdt = ml_dtypes.bfloat16 if h_dtype == "bf16" else np.float32
    h = np.asarray(hidden_states, dtype=np.float32)
    cl = np.asarray(clause_len).astype(np.int64)
    B, T, D = h.shape
    assert D == DCH * P and B == NCORES * DPC
    starts = np.cumsum(cl, axis=1) - cl                       # [B, J]
    need = int((starts[:, -1] + K).max())
    T_eff = -(-need // QN) * QN
    T_copy = min(T_eff, T)
    NT = DPC * T_eff

    fcb = float(np.asarray(fc_b).reshape(-1)[0])
    emb = float(np.asarray(emo_b).reshape(-1)[0])
    w2 = np.stack([np.asarray(fc_w, np.float32),
                   np.asarray(emo_w, np.float32)], axis=1)    # [768, 2]
    w2 = np.ascontiguousarray(w2.reshape(DCH, P, 2)).astype(np_hdt)

    tokk = np.arange(K)
    in_maps = []
    for c in range(NCORES):
        docs = slice(c * DPC, (c + 1) * DPC)
        hc = h[docs]                                          # [DPC, T, D]
        ht = np.zeros((D, DPC, T_eff), np_hdt)
        ht[:, :, :T_copy] = hc[:, :T_copy, :].transpose(2, 0, 1).astype(np_hdt)
        ht = np.ascontiguousarray(ht).reshape(DCH, P, NT)

        st = starts[docs]                                     # [DPC, J]
        lc = cl[docs]
        w = np.arange(2 * P)
        b_l, j_l = w // J, w % J
        woff = (b_l * T_eff + st[b_l, j_l]).astype(np.int32).reshape(2, P, 1)
        mask = np.where(tokk[None, :] < lc[b_l, j_l][:, None],
                        np.float32(fcb), np.float32(NEG)).astype(np.float32)
        maskadd = mask.reshape(2, P, K)
        in_maps.append({"ht": ht, "w2": w2, "woff": woff, "maskadd": maskadd})
    return in_maps, NT, fcb, emb


def run(inputs, trace=False, h_dtype=H_DTYPE):
    in_maps, NT, fcb, emb = _prepare(**inputs, h_dtype=h_dtype)
    nc = bacc.Bacc(
        "TRN2", target_bir_lowering=False, debug=False, num_devices=NCORES
    )
    _emit_kernel(nc, NT, fcb, emb, h_dtype)
    nc.compile()
    res = run_bass_kernel_spmd(nc, in_maps, core_ids=list(range(NCORES)),
                               trace=trace)
    pred = np.concatenate(
        [r["out"].reshape(2 * P).reshape(DPC, J) for r in res.results], axis=0)
    return pred.astype(np.float32), res


def kernel(**inputs):
    pred, _ = run(inputs, trace=False)
    return pred
